# revision 30
# baseline (speedup 1.0000x reference)
"""Tensor-parallel GQA attention kernel for one TRN2 chip (8 NeuronCores).

Problem: hidden [1, 2048, 4096] -> q/k/v proj -> interleaved RoPE -> causal
GQA attention (32 q heads, 8 kv heads, head_dim 128) -> o_proj -> [1, 2048, 4096].

Sharding: tensor-parallel over heads. Core c owns q heads 4c..4c+3 and kv
head c. Attention output moves head-shard -> sequence-shard via AllToAll;
each core then runs the full o_proj for its 256 rows (no reduction).

Row mapping (per core r, HALF-A2A scheme): rows are split in two 128-row
groups so each head's AllToAll can be fired in two halves:
  half-1 rows = seq[(r//4)*512 + (r%4)*128 : +128]   (from chunks 0/1)
  half-2 rows = seq[(2+r//4)*512 + (r%4)*128 : +128] (from chunks 2/3)
The half-1 A2As fire right after chunk 1's attention (hidden under chunks
2-3); half-2 after chunk 3. o_proj head h stq=0 needs only half-1, so the
PE rolls from attention into o_proj with no collective wait.

Device scheme (everything transposed, [feature, seq]):
  - qT/kT computed as [d, s] via matmul(lhsT=W_tile, rhs=hT_tile); RoPE in
    rotate-half form (wq/wk columns de-interleaved on host; dot products
    are invariant to the shared permutation of q and k). 1/sqrt(dh) is
    folded into wq on host.
  - wq for all 4 local heads is RESIDENT in SBUF (loaded once at startup)
    instead of streamed per (chunk, head): saves 12.5 MB of HBM traffic,
    freeing DMA bandwidth for ht prefetch and the wo stream.
  - scoresT [t, s] = matmul(lhsT=kT_tile, rhs=q_chunk); Exp on ScalarE
    evacuates PSUM->SBUF bf16; causal masking via a 0/1 bf16 mask applied
    post-exp on the straddling diagonal t-tile.
  - attn@v accumulates [d, s] with lhsT=v_tile (natural [s, d] layout via
    PE transpose of vT); softmax sums via pairwise DVE adds + an all-ones
    PE matmul for the partition reduce; one division on the [d, s] output.
  - o_proj is pipelined per head; 8-rank psum chains rotate across two
    PSUM pools (4 chains in flight) so the DVE evacuation never gates a
    chain start. wo streams through a 13-buf SBUF pool as [128, 1024]
    tiles in (h, colgrp, rank) order so the DMA prefetches during
    attention. ao (A2A output) loads go on the GpSimd queue so their
    collective waits never block the SP's wo stream.
"""

import sys

if "/opt/trn_rl_repo" not in sys.path:
    sys.path.insert(0, "/opt/trn_rl_repo")

import numpy as np
import ml_dtypes

import concourse.bass as bass
import concourse.bacc as bacc
import concourse.mybir as mybir
import concourse.tile as tile
from concourse import bass_utils
from concourse.masks import make_identity

F32 = mybir.dt.float32
BF16 = mybir.dt.bfloat16
NPBF16 = ml_dtypes.bfloat16

S = 2048          # sequence length
HID = 4096        # hidden size
NH = 32           # q heads
NKV = 8           # kv heads
DH = 128          # head dim
G = NH // NKV     # q heads per kv head (= per core)
NCORES = 8
SC = S // NCORES  # seq rows per core after A2A (= 256)
CH = 512          # attention s-chunk width
NCH = S // CH     # 4 chunks
KT = HID // 128   # 32 hidden k-tiles
NCG = HID // 512  # 8 o_proj column groups

_CACHED = {}


def build_kernel():
    nc = bacc.Bacc("TRN2", target_bir_lowering=False, debug=False,
                   num_devices=NCORES)

    # ht arranged [scb][128][KT*CH] on host (partition-major so DMA lines are
    # 16 KB contiguous, not 1 KB gathers); weights pre-arranged to SBUF layouts
    ht_d = nc.declare_dram_parameter("ht", [NCH, 128, KT * CH], BF16,
                                     isOutput=False)
    wq_d = nc.declare_dram_parameter("wq", [G, 128, KT * 128], BF16, isOutput=False)
    wk_d = nc.declare_dram_parameter("wk", [128, KT * 128], BF16, isOutput=False)
    wv_d = nc.declare_dram_parameter("wv", [128, KT * 128], BF16, isOutput=False)
    # wo tiles in stream order (h, col-quarter, rank): [G, 4, NCORES, 128, 1024]
    wo_d = nc.declare_dram_parameter("wo", [G, 4, NCORES, 128, 1024], BF16,
                                     isOutput=False)
    cos2_d = nc.declare_dram_parameter("cos2", [128, S], BF16, isOutput=False)
    sins_d = nc.declare_dram_parameter("sins", [128, S], BF16, isOutput=False)
    # 0/1 bf16 mask for the straddling diagonal t-tile
    mask_d = nc.declare_dram_parameter("mask", [4, 128, CH], BF16, isOutput=False)
    # out as [stq, quad, 128, 1024] so each store writes 2 KB contiguous
    # per-partition lines (one [128, 1024] tile per (stq, quad))
    out_d = nc.declare_dram_parameter("out", [2, 4, 128, 1024], BF16,
                                      isOutput=True)

    from contextlib import ExitStack

    with tile.TileContext(nc) as tc:
        with ExitStack() as stack:
            ep = stack.enter_context
            constp = ep(tc.tile_pool(name="const", bufs=1))
            oaccp = ep(tc.tile_pool(name="oaccp", bufs=1))
            aop = ep(tc.tile_pool(name="aop", bufs=4))
            dramp = ep(tc.tile_pool(name="dram", bufs=1, space="DRAM"))
            htp = ep(tc.tile_pool(name="htp", bufs=3))
            wp = ep(tc.tile_pool(name="wp", bufs=2))
            wqrp = ep(tc.tile_pool(name="wqrp", bufs=1))
            kvp = ep(tc.tile_pool(name="kvp", bufs=1))
            psA = ep(tc.tile_pool(name="psA", bufs=2, space="PSUM"))
            ropep = ep(tc.tile_pool(name="ropep", bufs=2))
            qcp = ep(tc.tile_pool(name="qcp", bufs=2))
            cscp = ep(tc.tile_pool(name="cscp", bufs=2))
            psB = ep(tc.tile_pool(name="psB", bufs=2, space="PSUM"))
            psBo = ep(tc.tile_pool(name="psBo", bufs=2, space="PSUM"))
            psC = ep(tc.tile_pool(name="psC", bufs=2, space="PSUM"))
            probp = ep(tc.tile_pool(name="probp", bufs=4))
            smallp = ep(tc.tile_pool(name="smallp", bufs=2))
            rcp = ep(tc.tile_pool(name="rcp", bufs=1))
            aoutp = ep(tc.tile_pool(name="aoutp", bufs=2))
            wop = ep(tc.tile_pool(name="wop", bufs=20))
            outp = ep(tc.tile_pool(name="outp", bufs=2))
            # per-head half A2As: half 1 = rows from chunks 0/1 (fired after
            # chunk 1), half 2 = rows from chunks 2/3 (fired after chunk 3)
            a2a_in1 = [dramp.tile([NCORES, 1, 128, 128], BF16, name=f"a1i{p}")
                       for p in range(G)]
            a2a_out1 = [dramp.tile([NCORES, 1, 128, 128], BF16, name=f"a1o{p}")
                        for p in range(G)]
            a2a_in2 = [dramp.tile([NCORES, 1, 128, 128], BF16, name=f"a2i{p}")
                       for p in range(G)]
            a2a_out2 = [dramp.tile([NCORES, 1, 128, 128], BF16, name=f"a2o{p}")
                        for p in range(G)]

            _htn = [0]
            ht_last_dma = {}
            HKT = KT // 2  # k-tiles per half tile

            def ht_sub(t, scb, b, nb=4):
                # load the b-th of nb sub-blocks of chunk scb into half
                # tiles t = (A, B); sub-block = KT/nb consecutive k-tiles
                kb = KT // nb
                half = t[(b * kb) // HKT]
                off = (b * kb) % HKT
                return nc.sync.dma_start(
                    half[:, off * CH:(off + kb) * CH],
                    ht_d[scb, :, b * kb * CH:(b + 1) * kb * CH])

            def alloc_ht(scb):
                _htn[0] += 1
                ta = htp.tile([128, HKT * CH], BF16,
                              name=f"ht{scb}a_{_htn[0]}", tag="ht")
                tb = htp.tile([128, HKT * CH], BF16,
                              name=f"ht{scb}b_{_htn[0]}", tag="ht")
                return (ta, tb)

            def load_ht(scb):
                t = alloc_ht(scb)
                for b in range(4):
                    inst = ht_sub(t, scb, b)
                ht_last_dma[scb] = inst
                return t

            def ht_kt(t, kt):
                return t[kt // HKT][:, (kt % HKT) * CH:(kt % HKT + 1) * CH]

            def load_cs(scb):
                cosc = cscp.tile([128, CH], BF16, tag="cosc")
                nc.sync.dma_start(cosc[:], cos2_d[:, scb * CH:(scb + 1) * CH])
                sinc = cscp.tile([128, CH], BF16, tag="sinc")
                nc.sync.dma_start(sinc[:], sins_d[:, scb * CH:(scb + 1) * CH])
                return cosc, sinc

            def proj_group(w_t, ht_t):
                """One [128, 512] psum accumulating W_tile.T @ h-chunk."""
                ps = psA.tile([128, CH], F32, tag="proj")
                for kt in range(KT):
                    nc.tensor.matmul(
                        ps[:], w_t[:, kt * 128:(kt + 1) * 128],
                        ht_kt(ht_t, kt),
                        start=(kt == 0), stop=(kt == KT - 1))
                return ps

            def rope_evac(ps, dst_slice, cosc, sinc):
                # dst[0:64] = x1*c - x2*s ; dst[64:128] = x1*s + x2*c
                # cos2 = [c; c], sins = [s; -s]; bf16 for DVE 4x mode
                qf = ropep.tile([128, CH], BF16, tag="qf")
                nc.vector.tensor_copy(qf[:], ps[:])
                ra = ropep.tile([128, CH], BF16, tag="ra")
                nc.vector.tensor_tensor(ra[:], qf[:], cosc[:],
                                        op=mybir.AluOpType.mult)
                rb = ropep.tile([128, CH], BF16, tag="rb")
                nc.vector.tensor_tensor(rb[0:64, :], qf[64:128, :],
                                        sinc[64:128, :],
                                        op=mybir.AluOpType.mult)
                nc.vector.tensor_tensor(rb[64:128, :], qf[0:64, :],
                                        sinc[0:64, :],
                                        op=mybir.AluOpType.mult)
                nc.vector.tensor_tensor(dst_slice, ra[:], rb[:],
                                        op=mybir.AluOpType.add)

            _chain = [0]

            def o_proj_head(h, ao_h):
                """Head h's contribution to the local [SC, HID] output.

                Column-quarter blocks keep the wo working set small so the
                pool double-buffers block k+1's DMA under block k's matmuls.
                PSUM chains rotate across psC/psB (4 in flight) so the DVE
                oacc evacuation never gates the next chain start.
                """
                for quad in range(4):
                    wts = []
                    for r in range(NCORES):
                        wt = wop.tile([128, 1024], BF16, tag="wo")
                        inst = nc.sync.dma_start(wt[:], wo_d[h, quad, r])
                        if h == 0 and quad < 2 and 2 in ht_last_dma:
                            # order the dependency-free first wo wave behind
                            # the chunk-2 ht load: late enough to keep the
                            # startup DMA window clear, early enough that it
                            # drains before the late-phase control DMAs
                            tile.add_dep_helper(
                                inst.ins, ht_last_dma[2].ins, sync=False,
                                reason="wo prefetch after startup loads")
                        wts.append(wt)
                    ots = {}
                    for hc in range(2):
                        cg = quad * 2 + hc
                        for stq in range(2):
                            pool, tag = [(psC, "o"), (psB, "sc"),
                                         (psBo, "att")][_chain[0] % 3]
                            _chain[0] += 1
                            ps = pool.tile([128, 512], F32, tag=tag)
                            for r in range(NCORES):
                                nc.tensor.matmul(
                                    ps[:],
                                    ao_h[:, r * SC + stq * 128:
                                         r * SC + (stq + 1) * 128],
                                    wts[r][:, hc * 512:(hc + 1) * 512],
                                    start=(r == 0), stop=(r == NCORES - 1))
                            oi = stq * NCG + cg
                            if h == 0:
                                nc.vector.tensor_copy(oacc[oi][:], ps[:])
                            elif h < G - 1:
                                nc.vector.tensor_tensor(
                                    oacc[oi][:], oacc[oi][:], ps[:],
                                    op=mybir.AluOpType.add)
                            else:
                                if stq not in ots:
                                    ots[stq] = outp.tile(
                                        [128, 1024], BF16, tag="ot",
                                        name=f"ot{quad}_{stq}")
                                ot = ots[stq]
                                nc.vector.tensor_tensor(
                                    ot[:, hc * 512:(hc + 1) * 512],
                                    oacc[oi][:], ps[:],
                                    op=mybir.AluOpType.add)
                                if hc == 1:
                                    # store from the ScalarE queue (idle
                                    # during o_proj) so it never blocks the
                                    # SP wo stream
                                    nc.scalar.dma_start(
                                        out_d[stq, quad], ot[:])

            # ---- PE warm-up: dummy matmuls fill the startup DMA wait so the
            # HAM clock gate opens (1.2 -> 2.4 GHz) before real work lands;
            # results go to a scratch psum that is never read ----
            dums = constp.tile([128, 128], BF16)
            nc.vector.memset(dums[:], 0.0)
            ps_w = psA.tile([128, 128], F32, tag="proj")
            for i in range(100):
                nc.tensor.matmul(ps_w[:], dums[:], dums[:],
                                 start=(i == 0), stop=(i == 99))

            # ---- startup order: what MM #1 needs first (wk + ht block 0),
            # interleaved finely so the loads fan out across DMA queues ----
            wk_t = wp.tile([128, KT * 128], BF16, tag="w")
            ht_pre = alloc_ht(0)
            for b in range(4):
                nc.sync.dma_start(
                    wk_t[:, b * 8 * 128:(b + 1) * 8 * 128],
                    wk_d[:, b * 8 * 128:(b + 1) * 8 * 128])
                ht_sub(ht_pre, 0, 2 * b, nb=16)
                ht_sub(ht_pre, 0, 2 * b + 1, nb=16)
            # wv / rope tables / wq / masks go on the ScalarE DMA queue:
            # a second trigger ring that runs in parallel with the SP's
            # (which is saturated by wk + ht), and starts issuing earlier
            wv_t = wp.tile([128, KT * 128], BF16, tag="w")
            nc.scalar.dma_start(wv_t[:, 0:16 * 128], wv_d[:, 0:16 * 128])
            nc.scalar.dma_start(wv_t[:, 16 * 128:], wv_d[:, 16 * 128:])
            cos0 = cscp.tile([128, CH], BF16, tag="cosc", name="cos0")
            nc.scalar.dma_start(cos0[:], cos2_d[:, 0:CH])
            sin0 = cscp.tile([128, CH], BF16, tag="sinc", name="sin0")
            nc.scalar.dma_start(sin0[:], sins_d[:, 0:CH])
            for b in range(4, 8):
                inst = ht_sub(ht_pre, 0, b, nb=8)
            ht_last_dma[0] = inst
            kT = kvp.tile([128, S], BF16)
            v_sb = kvp.tile([128, S], BF16)  # s-tile st at cols [st*128, ..)
            wq_res = []
            for h in range(G):
                wq_t = wqrp.tile([128, KT * 128], BF16, tag=f"wq{h}",
                                 name=f"wq{h}")
                nc.scalar.dma_start(wq_t[:], wq_d[h])
                wq_res.append(wq_t)
            # only the (p <= c) diagonal mask tile is needed
            masks = constp.tile([128, 128], BF16)
            nc.scalar.dma_start(masks[:], mask_d[0, :, 0:128])
            ident = constp.tile([128, 128], BF16)
            make_identity(nc, ident[:])
            # all-ones f32 tile: PE reduce+broadcast of softmax denominators
            ones_t = constp.tile([128, 128], F32)
            nc.any.memset(ones_t[:], 1.0)

            # tiny A2A barrier: aligns core skew early so the first real
            # AllToAll's peer-wait is short; engines are untouched
            bar_in = dramp.tile([NCORES, 1, 1, 64], BF16, name="barin")
            bar_out = dramp.tile([NCORES, 1, 1, 64], BF16, name="barout")
            nc.gpsimd.collective_compute(
                "AllToAll", mybir.AluOpType.bypass,
                replica_groups=[list(range(NCORES))],
                ins=[bar_in.opt()], outs=[bar_out.opt()])

            # o_proj SBUF accumulator: 16 tiles [128, 512] bf16 (2 MB)
            oacc = [oaccp.tile([128, 512], BF16, name=f"oacc{i}",
                               tag=f"oacc{i}")
                    for i in range(2 * NCG)]

            # ---- scb-outer: k/v proj, then per-head q proj + attention ----
            # ht/cos/sin for chunk j+1 are issued at the TOP of chunk j, so
            # the SP queue reaches their triggers a full chunk early; the
            # pool-slot waits (benign: only o_sb quarter DMAs with a full
            # chunk of slack queue behind them) gate the actual transfers.
            ao_hs = []
            ht_nxt = cs_nxt = None
            for scb in range(NCH):
                if scb == 0:
                    ht_t, cosc, sinc = ht_pre, cos0, sin0
                else:
                    ht_t = ht_nxt
                    cosc, sinc = cs_nxt
                if scb + 1 < NCH:
                    ht_nxt = load_ht(scb + 1)
                    cs_nxt = load_cs(scb + 1)
                j = scb
                nt = (CH // 128) * (j + 1)

                # k chunk + rope
                ps_k = proj_group(wk_t, ht_t)
                rope_evac(ps_k, kT[:, scb * CH:(scb + 1) * CH], cosc, sinc)

                # v chunk: vT then PE-transpose to natural [s, d]
                ps = proj_group(wv_t, ht_t)
                vT_sb = ropep.tile([128, CH], BF16, tag="ra")
                nc.vector.tensor_copy(vT_sb[:], ps[:])
                for q4 in range(CH // 128):
                    st = scb * (CH // 128) + q4
                    ps_tr = psA.tile([128, 128], BF16, tag="proj")
                    nc.tensor.transpose(
                        ps_tr[:], vT_sb[:, q4 * 128:(q4 + 1) * 128],
                        ident[:])
                    nc.vector.tensor_copy(
                        v_sb[:, st * 128:(st + 1) * 128], ps_tr[:])

                # q proj for head 0 of this chunk (dense; nothing to overlap)
                ps_q = proj_group(wq_res[0], ht_t)
                qc = qcp.tile([128, CH], BF16, tag="qc")
                rope_evac(ps_q, qc[:], cosc, sinc)

                for h in range(G):
                    # next head's q-proj matmuls become PE filler inside this
                    # head's attention (hides the exp latency); its rope runs
                    # before this attention ends so qc_next is ready in time.
                    filler = []
                    ps_n = None
                    if h + 1 < G:
                        ps_n = psA.tile([128, CH], F32, tag="proj")

                        def mk(kt, ps_n=ps_n, wq_n=wq_res[h + 1], ht_t=ht_t):
                            def go():
                                nc.tensor.matmul(
                                    ps_n[:],
                                    wq_n[:, kt * 128:(kt + 1) * 128],
                                    ht_kt(ht_t, kt),
                                    start=(kt == 0), stop=(kt == KT - 1))
                            return go
                        filler = [mk(kt) for kt in range(KT)]
                    fit = iter(filler)

                    def fill(n):
                        for _ in range(n):
                            f = next(fit, None)
                            if f is None:
                                return
                            f()
                    per = (-(-len(filler) // max(nt - 3, 1))
                           if filler else 0)

                    # attention chunk (h, j)
                    att_ps = psBo.tile([128, CH], F32, tag="att")
                    acc0 = smallp.tile([128, CH], F32, tag="acc0")
                    nd = 4 * j  # full (non-diagonal) t-tiles
                    prs = []
                    for tt in range(nd):
                        sc = psB.tile([128, CH], F32, tag="sc")
                        nc.tensor.matmul(sc[:],
                                         kT[:, tt * 128:(tt + 1) * 128],
                                         qc[:], start=True, stop=True)
                        pr = probp.tile([128, CH], BF16, tag="pr")
                        nc.scalar.activation(
                            pr[:], sc[:], mybir.ActivationFunctionType.Exp)
                        nc.tensor.matmul(att_ps[:],
                                         v_sb[:, tt * 128:(tt + 1) * 128],
                                         pr[:],
                                         start=(tt == 0), stop=False)
                        fill(per)
                        # softmax-denominator: pairwise bf16 sums,
                        # then one f32 chain add per pair
                        prs.append(pr)
                        if tt % 2 == 1:
                            p0, p1 = prs[-2], prs[-1]
                            pp = smallp.tile([128, CH], BF16, tag="pp")
                            nc.vector.tensor_tensor(
                                pp[:], p0[:], p1[:], op=mybir.AluOpType.add)
                            if tt == 1:
                                nc.vector.tensor_copy(acc0[:], pp[:])
                            else:
                                nc.vector.tensor_tensor(
                                    acc0[:], acc0[:], pp[:],
                                    op=mybir.AluOpType.add)
                    # diagonal 512x512 block: each t-tile dt only attends
                    # s-cols >= dt*128, so compute at exact width w
                    for dt in range(4):
                        tt = nd + dt
                        c0 = dt * 128
                        w = CH - c0
                        sc = psB.tile([128, CH], F32, tag="sc")
                        nc.tensor.matmul(sc[:, 0:w],
                                         kT[:, tt * 128:(tt + 1) * 128],
                                         qc[:, c0:CH], start=True, stop=True)
                        pr = probp.tile([128, CH], BF16, tag="pr")
                        nc.scalar.activation(
                            pr[:, 0:w], sc[:, 0:w],
                            mybir.ActivationFunctionType.Exp)
                        # only the first 128 cols straddle the diagonal
                        nc.vector.tensor_tensor(
                            pr[:, 0:128], pr[:, 0:128], masks[:],
                            op=mybir.AluOpType.mult)
                        nc.tensor.matmul(att_ps[:, c0:CH],
                                         v_sb[:, tt * 128:(tt + 1) * 128],
                                         pr[:, 0:w],
                                         start=(tt == 0), stop=(dt == 3))
                        fill(per)
                        if dt == 0 and j == 0:
                            nc.vector.tensor_copy(acc0[:], pr[:])
                        else:
                            nc.vector.tensor_tensor(
                                acc0[:, c0:CH], acc0[:, c0:CH], pr[:, 0:w],
                                op=mybir.AluOpType.add)
                    # flush any remaining filler + rope the next head's q
                    fill(KT)
                    qc_next = None
                    if ps_n is not None:
                        qc_next = qcp.tile([128, CH], BF16, tag="qc")
                        rope_evac(ps_n, qc_next[:], cosc, sinc)
                    # sum over t-partitions + broadcast: all-ones fp32 matmul
                    sums_ps = psB.tile([128, CH], F32, tag="sc")
                    nc.tensor.matmul(
                        sums_ps[:], ones_t[:], acc0[:],
                        start=True, stop=True)
                    rc = rcp.tile([128, CH], F32, tag="rc")
                    nc.vector.reciprocal_approx_fast(out=rc[:], in_=sums_ps[:])
                    o_sb = aoutp.tile([128, CH], BF16, tag="o")
                    nc.vector.tensor_tensor(o_sb[:], att_ps[:], rc[:],
                                            op=mybir.AluOpType.mult)
                    # chunk j's 4 column-quarters go to 4 destination cores:
                    # j in {0,1} -> cores 4j..4j+3 (their half-1 rows);
                    # j in {2,3} -> cores 4(j-2)..4(j-2)+3 (half-2 rows)
                    tgt = a2a_in1[h] if j < 2 else a2a_in2[h]
                    jj = j % 2
                    nc.sync.dma_start(
                        tgt[4 * jj:4 * jj + 4, 0].rearrange(
                            "r p c -> p r c"),
                        o_sb.rearrange("p (r c) -> p r c", r=4))
                    if scb == 1:
                        nc.gpsimd.collective_compute(
                            "AllToAll", mybir.AluOpType.bypass,
                            replica_groups=[list(range(NCORES))],
                            ins=[a2a_in1[h].opt()],
                            outs=[a2a_out1[h].opt()])
                    elif scb == NCH - 1:
                        nc.gpsimd.collective_compute(
                            "AllToAll", mybir.AluOpType.bypass,
                            replica_groups=[list(range(NCORES))],
                            ins=[a2a_in2[h].opt()],
                            outs=[a2a_out2[h].opt()])
                        # interleave the half-2 ao load right after its
                        # trigger: gpsimd blocks on A2A(h,2) completion,
                        # which lands before the next head's trigger input
                        # is ready anyway
                        ao4 = ao_hs[h].rearrange(
                            "p (r t c) -> p r t c", r=NCORES, t=2)
                        nc.gpsimd.dma_start(
                            ao4[:, :, 1, :],
                            a2a_out2[h][:, 0].rearrange("r p c -> p r c"))
                    qc = qc_next

                if scb == 1:
                    # half-1 ao loads on the GpSimd queue, emitted after the
                    # last half-1 trigger so they run mid-attention and
                    # never block the SP's wo/ht streams; ao_h cols
                    # r*256+[0:128] = half-1, r*256+[128:256] = half-2
                    for h in range(G):
                        ao_h = aop.tile([128, NCORES * SC], BF16, tag="ao",
                                        name=f"ao{h}")
                        ao4 = ao_h.rearrange(
                            "p (r t c) -> p r t c", r=NCORES, t=2)
                        nc.gpsimd.dma_start(
                            ao4[:, :, 0, :],
                            a2a_out1[h][:, 0].rearrange("r p c -> p r c"))
                        ao_hs.append(ao_h)

            # o_proj per head, created after all attention so the scheduler
            # keeps late-head attention ahead of early-head o_proj
            for h in range(G):
                o_proj_head(h, ao_hs[h])

    nc.compile()
    return nc


def _deinterleave(w):
    # per 128-col head block: [even cols, odd cols]
    hid, cols = w.shape
    nh = cols // DH
    w = w.reshape(hid, nh, DH)
    w = np.concatenate([w[:, :, 0::2], w[:, :, 1::2]], axis=2)
    return w.reshape(hid, cols)


def _prep_inputs(hidden_states, cos, sin, position_ids, attention_mask,
                 wq, wk, wv, wo):
    h = np.asarray(hidden_states, dtype=np.float32)[0]          # [S, HID]
    ht = np.ascontiguousarray(h.T)                              # [HID, S]
    # [NCH, 128(p), KT*CH]: ht4[scb, p, kt*CH + c] = ht[kt*128 + p, scb*CH + c]
    ht4 = np.ascontiguousarray(
        ht.reshape(KT, 128, NCH, CH).transpose(2, 1, 0, 3).reshape(
            NCH, 128, KT * CH)).astype(NPBF16)

    pos = np.asarray(position_ids)[0].astype(np.int64)
    ct = np.asarray(cos, dtype=np.float32)[pos].T               # [64, S]
    st = np.asarray(sin, dtype=np.float32)[pos].T
    cos2 = np.ascontiguousarray(np.concatenate([ct, ct], axis=0)).astype(NPBF16)
    sins = np.ascontiguousarray(np.concatenate([st, -st], axis=0)).astype(NPBF16)

    scale = 1.0 / np.sqrt(np.float32(DH))
    wq_p = (_deinterleave(np.asarray(wq, dtype=np.float32)) * scale)
    wk_p = _deinterleave(np.asarray(wk, dtype=np.float32))
    wv_p = np.asarray(wv, dtype=np.float32)
    # wo -> [G(h), 4, NCORES(r), 128, 1024]; tile (h, quad, r) = rows of
    # global head g = r*G + h, cols [quad*1024, (quad+1)*1024)
    wo4 = np.asarray(wo, dtype=np.float32).reshape(NCORES, G, 128, 4, 1024)
    wo_p = np.ascontiguousarray(wo4.transpose(1, 3, 0, 2, 4)).astype(NPBF16)

    # 0/1 bf16 masks for diagonal t-tiles: mask_i[p, c] = (p + 128*i <= c)
    p = np.arange(128)[:, None]
    c = np.arange(CH)[None, :]
    mask = np.stack([(p + 128 * i <= c) for i in range(4)]).astype(NPBF16)

    in_maps = []
    for core in range(NCORES):
        wq_c = wq_p[:, core * G * DH:(core + 1) * G * DH]       # [HID, 512]
        # -> [G, 128(p), KT*128] matching the SBUF tile layout
        wq_c = np.ascontiguousarray(
            wq_c.reshape(KT, 128, G, DH).transpose(2, 1, 0, 3).reshape(
                G, 128, KT * DH)).astype(NPBF16)
        wk_c = np.ascontiguousarray(
            wk_p[:, core * DH:(core + 1) * DH].reshape(KT, 128, DH)
            .transpose(1, 0, 2).reshape(128, KT * DH)).astype(NPBF16)
        wv_c = np.ascontiguousarray(
            wv_p[:, core * DH:(core + 1) * DH].reshape(KT, 128, DH)
            .transpose(1, 0, 2).reshape(128, KT * DH)).astype(NPBF16)
        in_maps.append({
            "ht": ht4, "wq": wq_c, "wk": wk_c, "wv": wv_c, "wo": wo_p,
            "cos2": cos2, "sins": sins, "mask": mask,
        })
    return in_maps


def kernel(hidden_states, cos, sin, position_ids, attention_mask,
           wq, wk, wv, wo, **run_kwargs):
    if "nc" not in _CACHED:
        _CACHED["nc"] = build_kernel()
    nc = _CACHED["nc"]
    in_maps = _prep_inputs(hidden_states, cos, sin, position_ids,
                           attention_mask, wq, wk, wv, wo)
    res = bass_utils.run_bass_kernel_spmd(
        nc, in_maps, core_ids=list(range(NCORES)), **run_kwargs)
    # core r's out rows: stq=0 -> seq[(r//4)*512 + (r%4)*128 : +128],
    #                    stq=1 -> seq[(2+r//4)*512 + (r%4)*128 : +128];
    # device layout is [stq, quad, 128, 1024]
    full = np.empty((S, HID), dtype=np.float32)
    for r in range(NCORES):
        o = np.asarray(res.results[r]["out"], dtype=np.float32)
        o = o.reshape(2, 4, 128, 1024).transpose(0, 2, 1, 3).reshape(
            2, 128, HID)
        b1 = (r // 4) * 512 + (r % 4) * 128
        b2 = (2 + r // 4) * 512 + (r % 4) * 128
        full[b1:b1 + 128] = o[0]
        full[b2:b2 + 128] = o[1]
    out = full.reshape(1, S, HID)
    if run_kwargs:
        _CACHED["last_result"] = res
    return out


# revision 32
# speedup vs baseline: 1.0332x; 1.0332x over previous
"""Tensor-parallel GQA attention kernel for one TRN2 chip (8 NeuronCores).

Problem: hidden [1, 2048, 4096] -> q/k/v proj -> interleaved RoPE -> causal
GQA attention (32 q heads, 8 kv heads, head_dim 128) -> o_proj -> [1, 2048, 4096].

Sharding: tensor-parallel over heads. Core c owns q heads 4c..4c+3 and kv
head c. Attention output moves head-shard -> sequence-shard via AllToAll;
each core then runs the full o_proj for its 256 rows (no reduction).

Row mapping (per core r, HALF-A2A scheme): rows are split in two 128-row
groups so each head's AllToAll can be fired in two halves:
  half-1 rows = seq[(r//4)*512 + (r%4)*128 : +128]   (from chunks 0/1)
  half-2 rows = seq[(2+r//4)*512 + (r%4)*128 : +128] (from chunks 2/3)
The half-1 A2As fire right after chunk 1's attention (hidden under chunks
2-3); half-2 after chunk 3. o_proj head h stq=0 needs only half-1, so the
PE rolls from attention into o_proj with no collective wait.

Device scheme (everything transposed, [feature, seq]):
  - qT/kT computed as [d, s] via matmul(lhsT=W_tile, rhs=hT_tile); RoPE in
    rotate-half form (wq/wk columns de-interleaved on host; dot products
    are invariant to the shared permutation of q and k). 1/sqrt(dh) is
    folded into wq on host.
  - wq for all 4 local heads is RESIDENT in SBUF (loaded once at startup)
    instead of streamed per (chunk, head): saves 12.5 MB of HBM traffic,
    freeing DMA bandwidth for ht prefetch and the wo stream.
  - scoresT [t, s] = matmul(lhsT=kT_tile, rhs=q_chunk); Exp on ScalarE
    evacuates PSUM->SBUF bf16; causal masking via a 0/1 bf16 mask applied
    post-exp on the straddling diagonal t-tile.
  - attn@v accumulates [d, s] with lhsT=v_tile (natural [s, d] layout via
    PE transpose of vT); softmax sums via pairwise DVE adds + an all-ones
    PE matmul for the partition reduce; one division on the [d, s] output.
  - o_proj is pipelined per head; 8-rank psum chains rotate across two
    PSUM pools (4 chains in flight) so the DVE evacuation never gates a
    chain start. wo streams through a 13-buf SBUF pool as [128, 1024]
    tiles in (h, colgrp, rank) order so the DMA prefetches during
    attention. ao (A2A output) loads go on the GpSimd queue so their
    collective waits never block the SP's wo stream.
"""

import sys

if "/opt/trn_rl_repo" not in sys.path:
    sys.path.insert(0, "/opt/trn_rl_repo")

import numpy as np
import ml_dtypes

import concourse.bass as bass
import concourse.bacc as bacc
import concourse.mybir as mybir
import concourse.tile as tile
from concourse import bass_utils
from concourse.masks import make_identity

F32 = mybir.dt.float32
BF16 = mybir.dt.bfloat16
NPBF16 = ml_dtypes.bfloat16

S = 2048          # sequence length
HID = 4096        # hidden size
NH = 32           # q heads
NKV = 8           # kv heads
DH = 128          # head dim
G = NH // NKV     # q heads per kv head (= per core)
NCORES = 8
SC = S // NCORES  # seq rows per core after A2A (= 256)
CH = 512          # attention s-chunk width
NCH = S // CH     # 4 chunks
KT = HID // 128   # 32 hidden k-tiles
NCG = HID // 512  # 8 o_proj column groups

_CACHED = {}


def build_kernel():
    nc = bacc.Bacc("TRN2", target_bir_lowering=False, debug=False,
                   num_devices=NCORES)

    # ht arranged [scb][128][KT*CH] on host (partition-major so DMA lines are
    # 16 KB contiguous, not 1 KB gathers); weights pre-arranged to SBUF layouts
    ht_d = nc.declare_dram_parameter("ht", [NCH, 128, KT * CH], BF16,
                                     isOutput=False)
    wq_d = nc.declare_dram_parameter("wq", [G, 128, KT * 128], BF16, isOutput=False)
    wk_d = nc.declare_dram_parameter("wk", [128, KT * 128], BF16, isOutput=False)
    wv_d = nc.declare_dram_parameter("wv", [128, KT * 128], BF16, isOutput=False)
    # wo tiles in stream order (h, col-quarter, rank): [G, 4, NCORES, 128, 1024]
    wo_d = nc.declare_dram_parameter("wo", [G, 4, NCORES, 128, 1024], BF16,
                                     isOutput=False)
    cos2_d = nc.declare_dram_parameter("cos2", [128, S], BF16, isOutput=False)
    sins_d = nc.declare_dram_parameter("sins", [128, S], BF16, isOutput=False)
    # 0/1 bf16 mask for the straddling diagonal t-tile
    mask_d = nc.declare_dram_parameter("mask", [4, 128, CH], BF16, isOutput=False)
    # out as [stq, quad, 128, 1024] so each store writes 2 KB contiguous
    # per-partition lines (one [128, 1024] tile per (stq, quad))
    out_d = nc.declare_dram_parameter("out", [2, 4, 128, 1024], BF16,
                                      isOutput=True)

    from contextlib import ExitStack

    with tile.TileContext(nc) as tc:
        with ExitStack() as stack:
            ep = stack.enter_context
            constp = ep(tc.tile_pool(name="const", bufs=1))
            oaccp = ep(tc.tile_pool(name="oaccp", bufs=1))
            aop = ep(tc.tile_pool(name="aop", bufs=4))
            dramp = ep(tc.tile_pool(name="dram", bufs=1, space="DRAM"))
            htp = ep(tc.tile_pool(name="htp", bufs=3))
            wp = ep(tc.tile_pool(name="wp", bufs=2))
            wqrp = ep(tc.tile_pool(name="wqrp", bufs=1))
            kvp = ep(tc.tile_pool(name="kvp", bufs=1))
            psA = ep(tc.tile_pool(name="psA", bufs=2, space="PSUM"))
            ropep = ep(tc.tile_pool(name="ropep", bufs=2))
            qcp = ep(tc.tile_pool(name="qcp", bufs=2))
            cscp = ep(tc.tile_pool(name="cscp", bufs=2))
            psB = ep(tc.tile_pool(name="psB", bufs=2, space="PSUM"))
            psBo = ep(tc.tile_pool(name="psBo", bufs=2, space="PSUM"))
            psC = ep(tc.tile_pool(name="psC", bufs=2, space="PSUM"))
            probp = ep(tc.tile_pool(name="probp", bufs=4))
            smallp = ep(tc.tile_pool(name="smallp", bufs=2))
            rcp = ep(tc.tile_pool(name="rcp", bufs=1))
            aoutp = ep(tc.tile_pool(name="aoutp", bufs=2))
            wop = ep(tc.tile_pool(name="wop", bufs=20))
            outp = ep(tc.tile_pool(name="outp", bufs=2))
            # per-head half A2As: half 1 = rows from chunks 0/1 (fired after
            # chunk 1), half 2 = rows from chunks 2/3 (fired after chunk 3)
            a2a_in1 = [dramp.tile([NCORES, 1, 128, 128], BF16, name=f"a1i{p}")
                       for p in range(G)]
            a2a_out1 = [dramp.tile([NCORES, 1, 128, 128], BF16, name=f"a1o{p}")
                        for p in range(G)]
            a2a_in2 = [dramp.tile([NCORES, 1, 128, 128], BF16, name=f"a2i{p}")
                       for p in range(G)]
            a2a_out2 = [dramp.tile([NCORES, 1, 128, 128], BF16, name=f"a2o{p}")
                        for p in range(G)]

            _htn = [0]
            ht_last_dma = {}
            HKT = KT // 2  # k-tiles per half tile

            def ht_sub(t, scb, b, nb=4):
                # load the b-th of nb sub-blocks of chunk scb into half
                # tiles t = (A, B); sub-block = KT/nb consecutive k-tiles
                kb = KT // nb
                half = t[(b * kb) // HKT]
                off = (b * kb) % HKT
                return nc.sync.dma_start(
                    half[:, off * CH:(off + kb) * CH],
                    ht_d[scb, :, b * kb * CH:(b + 1) * kb * CH])

            def alloc_ht(scb):
                _htn[0] += 1
                ta = htp.tile([128, HKT * CH], BF16,
                              name=f"ht{scb}a_{_htn[0]}", tag="ht")
                tb = htp.tile([128, HKT * CH], BF16,
                              name=f"ht{scb}b_{_htn[0]}", tag="ht")
                return (ta, tb)

            def load_ht(scb):
                t = alloc_ht(scb)
                for b in range(4):
                    inst = ht_sub(t, scb, b)
                ht_last_dma[scb] = inst
                return t

            def ht_kt(t, kt):
                return t[kt // HKT][:, (kt % HKT) * CH:(kt % HKT + 1) * CH]

            def load_cs(scb):
                cosc = cscp.tile([128, CH], BF16, tag="cosc")
                nc.sync.dma_start(cosc[:], cos2_d[:, scb * CH:(scb + 1) * CH])
                sinc = cscp.tile([128, CH], BF16, tag="sinc")
                nc.sync.dma_start(sinc[:], sins_d[:, scb * CH:(scb + 1) * CH])
                return cosc, sinc

            def proj_group(w_t, ht_t):
                """One [128, 512] psum accumulating W_tile.T @ h-chunk."""
                ps = psA.tile([128, CH], F32, tag="proj")
                for kt in range(KT):
                    nc.tensor.matmul(
                        ps[:], w_t[:, kt * 128:(kt + 1) * 128],
                        ht_kt(ht_t, kt),
                        start=(kt == 0), stop=(kt == KT - 1))
                return ps

            def rope_evac(ps, dst_slice, cosc, sinc):
                # dst[0:64] = x1*c - x2*s ; dst[64:128] = x1*s + x2*c
                # cos2 = [c; c], sins = [s; -s]; bf16 for DVE 4x mode
                qf = ropep.tile([128, CH], BF16, tag="qf")
                nc.vector.tensor_copy(qf[:], ps[:])
                ra = ropep.tile([128, CH], BF16, tag="ra")
                nc.vector.tensor_tensor(ra[:], qf[:], cosc[:],
                                        op=mybir.AluOpType.mult)
                rb = ropep.tile([128, CH], BF16, tag="rb")
                nc.vector.tensor_tensor(rb[0:64, :], qf[64:128, :],
                                        sinc[64:128, :],
                                        op=mybir.AluOpType.mult)
                nc.vector.tensor_tensor(rb[64:128, :], qf[0:64, :],
                                        sinc[0:64, :],
                                        op=mybir.AluOpType.mult)
                nc.vector.tensor_tensor(dst_slice, ra[:], rb[:],
                                        op=mybir.AluOpType.add)

            _chain = [0]

            def o_proj_head(h, ao_h):
                """Head h's contribution to the local [SC, HID] output.

                Column-quarter blocks keep the wo working set small so the
                pool double-buffers block k+1's DMA under block k's matmuls.
                PSUM chains rotate across psC/psB (4 in flight) so the DVE
                oacc evacuation never gates the next chain start.
                """
                for quad in range(4):
                    wts = []
                    for r in range(NCORES):
                        wt = wop.tile([128, 1024], BF16, tag="wo")
                        inst = nc.sync.dma_start(wt[:], wo_d[h, quad, r])
                        if h == 0 and quad < 2 and 2 in ht_last_dma:
                            # order the dependency-free first wo wave behind
                            # the chunk-2 ht load: late enough to keep the
                            # startup DMA window clear, early enough that it
                            # drains before the late-phase control DMAs
                            tile.add_dep_helper(
                                inst.ins, ht_last_dma[2].ins, sync=False,
                                reason="wo prefetch after startup loads")
                        wts.append(wt)
                    ots = {}
                    for hc in range(2):
                        cg = quad * 2 + hc
                        for stq in range(2):
                            pool, tag = [(psC, "o"), (psB, "sc"),
                                         (psBo, "att")][_chain[0] % 3]
                            _chain[0] += 1
                            ps = pool.tile([128, 512], F32, tag=tag)
                            for r in range(NCORES):
                                nc.tensor.matmul(
                                    ps[:],
                                    ao_h[:, r * SC + stq * 128:
                                         r * SC + (stq + 1) * 128],
                                    wts[r][:, hc * 512:(hc + 1) * 512],
                                    start=(r == 0), stop=(r == NCORES - 1))
                            oi = stq * NCG + cg
                            if h == 0:
                                nc.vector.tensor_copy(oacc[oi][:], ps[:])
                            elif h < G - 1:
                                nc.vector.tensor_tensor(
                                    oacc[oi][:], oacc[oi][:], ps[:],
                                    op=mybir.AluOpType.add)
                            else:
                                if stq not in ots:
                                    ots[stq] = outp.tile(
                                        [128, 1024], BF16, tag="ot",
                                        name=f"ot{quad}_{stq}")
                                ot = ots[stq]
                                nc.vector.tensor_tensor(
                                    ot[:, hc * 512:(hc + 1) * 512],
                                    oacc[oi][:], ps[:],
                                    op=mybir.AluOpType.add)
                                if hc == 1:
                                    # store from the ScalarE queue (idle
                                    # during o_proj) so it never blocks the
                                    # SP wo stream
                                    nc.scalar.dma_start(
                                        out_d[stq, quad], ot[:])

            # ---- PE warm-up: dummy matmuls fill the startup DMA wait so the
            # HAM clock gate opens (1.2 -> 2.4 GHz) before real work lands;
            # results go to a scratch psum that is never read ----
            dums = constp.tile([128, 128], BF16)
            nc.vector.memset(dums[:], 0.0)
            ps_w = psA.tile([128, 128], F32, tag="proj")
            for i in range(100):
                nc.tensor.matmul(ps_w[:], dums[:], dums[:],
                                 start=(i == 0), stop=(i == 99))

            # ---- startup order: what MM #1 needs first (wk + ht block 0),
            # interleaved finely so the loads fan out across DMA queues ----
            wk_t = wp.tile([128, KT * 128], BF16, tag="w")
            ht_pre = alloc_ht(0)
            for b in range(4):
                nc.sync.dma_start(
                    wk_t[:, b * 8 * 128:(b + 1) * 8 * 128],
                    wk_d[:, b * 8 * 128:(b + 1) * 8 * 128])
                ht_sub(ht_pre, 0, 2 * b, nb=16)
                ht_sub(ht_pre, 0, 2 * b + 1, nb=16)
            # interleave wv halves with ht half-B so each consumer stays
            # just ahead of the startup DMA stream
            # wv + chunk-0 rope tables (1.25 MB, needed by ~25us) go on the
            # ScalarE DMA queue: it starts issuing at ~3us while the SP ring
            # is saturated with wk+ht, and the small size steals negligible
            # HBM bandwidth from the ht stream (unlike wq, which regressed)
            wv_t = wp.tile([128, KT * 128], BF16, tag="w")
            nc.scalar.dma_start(wv_t[:, 0:16 * 128], wv_d[:, 0:16 * 128])
            nc.scalar.dma_start(wv_t[:, 16 * 128:], wv_d[:, 16 * 128:])
            cos0 = cscp.tile([128, CH], BF16, tag="cosc", name="cos0")
            nc.scalar.dma_start(cos0[:], cos2_d[:, 0:CH])
            sin0 = cscp.tile([128, CH], BF16, tag="sinc", name="sin0")
            nc.scalar.dma_start(sin0[:], sins_d[:, 0:CH])
            for b in range(4, 8):
                inst = ht_sub(ht_pre, 0, b, nb=8)
            ht_last_dma[0] = inst
            kT = kvp.tile([128, S], BF16)
            v_sb = kvp.tile([128, S], BF16)  # s-tile st at cols [st*128, ..)
            wq_res = []
            for h in range(G):
                wq_t = wqrp.tile([128, KT * 128], BF16, tag=f"wq{h}",
                                 name=f"wq{h}")
                nc.sync.dma_start(wq_t[:], wq_d[h])
                wq_res.append(wq_t)
            # only the (p <= c) diagonal mask tile is needed
            masks = constp.tile([128, 128], BF16)
            nc.sync.dma_start(masks[:], mask_d[0, :, 0:128])
            ident = constp.tile([128, 128], BF16)
            make_identity(nc, ident[:])
            # all-ones f32 tile: PE reduce+broadcast of softmax denominators
            ones_t = constp.tile([128, 128], F32)
            nc.any.memset(ones_t[:], 1.0)

            # tiny A2A barrier: aligns core skew early so the first real
            # AllToAll's peer-wait is short; engines are untouched
            bar_in = dramp.tile([NCORES, 1, 1, 64], BF16, name="barin")
            bar_out = dramp.tile([NCORES, 1, 1, 64], BF16, name="barout")
            nc.gpsimd.collective_compute(
                "AllToAll", mybir.AluOpType.bypass,
                replica_groups=[list(range(NCORES))],
                ins=[bar_in.opt()], outs=[bar_out.opt()])

            # o_proj SBUF accumulator: 16 tiles [128, 512] bf16 (2 MB)
            oacc = [oaccp.tile([128, 512], BF16, name=f"oacc{i}",
                               tag=f"oacc{i}")
                    for i in range(2 * NCG)]

            # ---- scb-outer: k/v proj, then per-head q proj + attention ----
            # ht/cos/sin for chunk j+1 are issued at the TOP of chunk j, so
            # the SP queue reaches their triggers a full chunk early; the
            # pool-slot waits (benign: only o_sb quarter DMAs with a full
            # chunk of slack queue behind them) gate the actual transfers.
            ao_hs = []
            ht_nxt = cs_nxt = None
            for scb in range(NCH):
                if scb == 0:
                    ht_t, cosc, sinc = ht_pre, cos0, sin0
                else:
                    ht_t = ht_nxt
                    cosc, sinc = cs_nxt
                if scb + 1 < NCH:
                    ht_nxt = load_ht(scb + 1)
                    cs_nxt = load_cs(scb + 1)
                j = scb
                nt = (CH // 128) * (j + 1)

                # k chunk + rope
                ps_k = proj_group(wk_t, ht_t)
                rope_evac(ps_k, kT[:, scb * CH:(scb + 1) * CH], cosc, sinc)

                # v chunk: vT then PE-transpose to natural [s, d]
                ps = proj_group(wv_t, ht_t)
                vT_sb = ropep.tile([128, CH], BF16, tag="ra")
                nc.vector.tensor_copy(vT_sb[:], ps[:])
                for q4 in range(CH // 128):
                    st = scb * (CH // 128) + q4
                    ps_tr = psA.tile([128, 128], BF16, tag="proj")
                    nc.tensor.transpose(
                        ps_tr[:], vT_sb[:, q4 * 128:(q4 + 1) * 128],
                        ident[:])
                    nc.vector.tensor_copy(
                        v_sb[:, st * 128:(st + 1) * 128], ps_tr[:])

                # q proj for head 0 of this chunk (dense; nothing to overlap)
                ps_q = proj_group(wq_res[0], ht_t)
                qc = qcp.tile([128, CH], BF16, tag="qc")
                rope_evac(ps_q, qc[:], cosc, sinc)

                for h in range(G):
                    # next head's q-proj matmuls become PE filler inside this
                    # head's attention (hides the exp latency); its rope runs
                    # before this attention ends so qc_next is ready in time.
                    filler = []
                    ps_n = None
                    if h + 1 < G:
                        ps_n = psA.tile([128, CH], F32, tag="proj")

                        def mk(kt, ps_n=ps_n, wq_n=wq_res[h + 1], ht_t=ht_t):
                            def go():
                                nc.tensor.matmul(
                                    ps_n[:],
                                    wq_n[:, kt * 128:(kt + 1) * 128],
                                    ht_kt(ht_t, kt),
                                    start=(kt == 0), stop=(kt == KT - 1))
                            return go
                        filler = [mk(kt) for kt in range(KT)]
                    fit = iter(filler)

                    def fill(n):
                        for _ in range(n):
                            f = next(fit, None)
                            if f is None:
                                return
                            f()
                    per = (-(-len(filler) // max(nt - 3, 1))
                           if filler else 0)

                    # attention chunk (h, j)
                    att_ps = psBo.tile([128, CH], F32, tag="att")
                    acc0 = smallp.tile([128, CH], F32, tag="acc0")
                    nd = 4 * j  # full (non-diagonal) t-tiles
                    prs = []
                    for tt in range(nd):
                        sc = psB.tile([128, CH], F32, tag="sc")
                        nc.tensor.matmul(sc[:],
                                         kT[:, tt * 128:(tt + 1) * 128],
                                         qc[:], start=True, stop=True)
                        pr = probp.tile([128, CH], BF16, tag="pr")
                        nc.scalar.activation(
                            pr[:], sc[:], mybir.ActivationFunctionType.Exp)
                        nc.tensor.matmul(att_ps[:],
                                         v_sb[:, tt * 128:(tt + 1) * 128],
                                         pr[:],
                                         start=(tt == 0), stop=False)
                        fill(per)
                        # softmax-denominator: pairwise bf16 sums,
                        # then one f32 chain add per pair
                        prs.append(pr)
                        if tt % 2 == 1:
                            p0, p1 = prs[-2], prs[-1]
                            pp = smallp.tile([128, CH], BF16, tag="pp")
                            nc.vector.tensor_tensor(
                                pp[:], p0[:], p1[:], op=mybir.AluOpType.add)
                            if tt == 1:
                                nc.vector.tensor_copy(acc0[:], pp[:])
                            else:
                                nc.vector.tensor_tensor(
                                    acc0[:], acc0[:], pp[:],
                                    op=mybir.AluOpType.add)
                    # diagonal 512x512 block: each t-tile dt only attends
                    # s-cols >= dt*128, so compute at exact width w
                    for dt in range(4):
                        tt = nd + dt
                        c0 = dt * 128
                        w = CH - c0
                        sc = psB.tile([128, CH], F32, tag="sc")
                        nc.tensor.matmul(sc[:, 0:w],
                                         kT[:, tt * 128:(tt + 1) * 128],
                                         qc[:, c0:CH], start=True, stop=True)
                        pr = probp.tile([128, CH], BF16, tag="pr")
                        nc.scalar.activation(
                            pr[:, 0:w], sc[:, 0:w],
                            mybir.ActivationFunctionType.Exp)
                        # only the first 128 cols straddle the diagonal
                        nc.vector.tensor_tensor(
                            pr[:, 0:128], pr[:, 0:128], masks[:],
                            op=mybir.AluOpType.mult)
                        nc.tensor.matmul(att_ps[:, c0:CH],
                                         v_sb[:, tt * 128:(tt + 1) * 128],
                                         pr[:, 0:w],
                                         start=(tt == 0), stop=(dt == 3))
                        fill(per)
                        if dt == 0 and j == 0:
                            nc.vector.tensor_copy(acc0[:], pr[:])
                        else:
                            nc.vector.tensor_tensor(
                                acc0[:, c0:CH], acc0[:, c0:CH], pr[:, 0:w],
                                op=mybir.AluOpType.add)
                    # flush any remaining filler + rope the next head's q
                    fill(KT)
                    qc_next = None
                    if ps_n is not None:
                        qc_next = qcp.tile([128, CH], BF16, tag="qc")
                        rope_evac(ps_n, qc_next[:], cosc, sinc)
                    # sum over t-partitions + broadcast: all-ones fp32 matmul
                    sums_ps = psB.tile([128, CH], F32, tag="sc")
                    nc.tensor.matmul(
                        sums_ps[:], ones_t[:], acc0[:],
                        start=True, stop=True)
                    rc = rcp.tile([128, CH], F32, tag="rc")
                    nc.vector.reciprocal_approx_fast(out=rc[:], in_=sums_ps[:])
                    o_sb = aoutp.tile([128, CH], BF16, tag="o")
                    nc.vector.tensor_tensor(o_sb[:], att_ps[:], rc[:],
                                            op=mybir.AluOpType.mult)
                    # chunk j's 4 column-quarters go to 4 destination cores:
                    # j in {0,1} -> cores 4j..4j+3 (their half-1 rows);
                    # j in {2,3} -> cores 4(j-2)..4(j-2)+3 (half-2 rows)
                    tgt = a2a_in1[h] if j < 2 else a2a_in2[h]
                    jj = j % 2
                    nc.sync.dma_start(
                        tgt[4 * jj:4 * jj + 4, 0].rearrange(
                            "r p c -> p r c"),
                        o_sb.rearrange("p (r c) -> p r c", r=4))
                    if scb == 1:
                        nc.gpsimd.collective_compute(
                            "AllToAll", mybir.AluOpType.bypass,
                            replica_groups=[list(range(NCORES))],
                            ins=[a2a_in1[h].opt()],
                            outs=[a2a_out1[h].opt()])
                    elif scb == NCH - 1:
                        nc.gpsimd.collective_compute(
                            "AllToAll", mybir.AluOpType.bypass,
                            replica_groups=[list(range(NCORES))],
                            ins=[a2a_in2[h].opt()],
                            outs=[a2a_out2[h].opt()])
                        # interleave the half-2 ao load right after its
                        # trigger: gpsimd blocks on A2A(h,2) completion,
                        # which lands before the next head's trigger input
                        # is ready anyway
                        ao4 = ao_hs[h].rearrange(
                            "p (r t c) -> p r t c", r=NCORES, t=2)
                        nc.gpsimd.dma_start(
                            ao4[:, :, 1, :],
                            a2a_out2[h][:, 0].rearrange("r p c -> p r c"))
                    qc = qc_next

                if scb == 1:
                    # half-1 ao loads on the GpSimd queue, emitted after the
                    # last half-1 trigger so they run mid-attention and
                    # never block the SP's wo/ht streams; ao_h cols
                    # r*256+[0:128] = half-1, r*256+[128:256] = half-2
                    for h in range(G):
                        ao_h = aop.tile([128, NCORES * SC], BF16, tag="ao",
                                        name=f"ao{h}")
                        ao4 = ao_h.rearrange(
                            "p (r t c) -> p r t c", r=NCORES, t=2)
                        nc.gpsimd.dma_start(
                            ao4[:, :, 0, :],
                            a2a_out1[h][:, 0].rearrange("r p c -> p r c"))
                        ao_hs.append(ao_h)

            # o_proj per head, created after all attention so the scheduler
            # keeps late-head attention ahead of early-head o_proj
            for h in range(G):
                o_proj_head(h, ao_hs[h])

    nc.compile()
    return nc


def _deinterleave(w):
    # per 128-col head block: [even cols, odd cols]
    hid, cols = w.shape
    nh = cols // DH
    w = w.reshape(hid, nh, DH)
    w = np.concatenate([w[:, :, 0::2], w[:, :, 1::2]], axis=2)
    return w.reshape(hid, cols)


def _prep_inputs(hidden_states, cos, sin, position_ids, attention_mask,
                 wq, wk, wv, wo):
    h = np.asarray(hidden_states, dtype=np.float32)[0]          # [S, HID]
    ht = np.ascontiguousarray(h.T)                              # [HID, S]
    # [NCH, 128(p), KT*CH]: ht4[scb, p, kt*CH + c] = ht[kt*128 + p, scb*CH + c]
    ht4 = np.ascontiguousarray(
        ht.reshape(KT, 128, NCH, CH).transpose(2, 1, 0, 3).reshape(
            NCH, 128, KT * CH)).astype(NPBF16)

    pos = np.asarray(position_ids)[0].astype(np.int64)
    ct = np.asarray(cos, dtype=np.float32)[pos].T               # [64, S]
    st = np.asarray(sin, dtype=np.float32)[pos].T
    cos2 = np.ascontiguousarray(np.concatenate([ct, ct], axis=0)).astype(NPBF16)
    sins = np.ascontiguousarray(np.concatenate([st, -st], axis=0)).astype(NPBF16)

    scale = 1.0 / np.sqrt(np.float32(DH))
    wq_p = (_deinterleave(np.asarray(wq, dtype=np.float32)) * scale)
    wk_p = _deinterleave(np.asarray(wk, dtype=np.float32))
    wv_p = np.asarray(wv, dtype=np.float32)
    # wo -> [G(h), 4, NCORES(r), 128, 1024]; tile (h, quad, r) = rows of
    # global head g = r*G + h, cols [quad*1024, (quad+1)*1024)
    wo4 = np.asarray(wo, dtype=np.float32).reshape(NCORES, G, 128, 4, 1024)
    wo_p = np.ascontiguousarray(wo4.transpose(1, 3, 0, 2, 4)).astype(NPBF16)

    # 0/1 bf16 masks for diagonal t-tiles: mask_i[p, c] = (p + 128*i <= c)
    p = np.arange(128)[:, None]
    c = np.arange(CH)[None, :]
    mask = np.stack([(p + 128 * i <= c) for i in range(4)]).astype(NPBF16)

    in_maps = []
    for core in range(NCORES):
        wq_c = wq_p[:, core * G * DH:(core + 1) * G * DH]       # [HID, 512]
        # -> [G, 128(p), KT*128] matching the SBUF tile layout
        wq_c = np.ascontiguousarray(
            wq_c.reshape(KT, 128, G, DH).transpose(2, 1, 0, 3).reshape(
                G, 128, KT * DH)).astype(NPBF16)
        wk_c = np.ascontiguousarray(
            wk_p[:, core * DH:(core + 1) * DH].reshape(KT, 128, DH)
            .transpose(1, 0, 2).reshape(128, KT * DH)).astype(NPBF16)
        wv_c = np.ascontiguousarray(
            wv_p[:, core * DH:(core + 1) * DH].reshape(KT, 128, DH)
            .transpose(1, 0, 2).reshape(128, KT * DH)).astype(NPBF16)
        in_maps.append({
            "ht": ht4, "wq": wq_c, "wk": wk_c, "wv": wv_c, "wo": wo_p,
            "cos2": cos2, "sins": sins, "mask": mask,
        })
    return in_maps


def kernel(hidden_states, cos, sin, position_ids, attention_mask,
           wq, wk, wv, wo, **run_kwargs):
    if "nc" not in _CACHED:
        _CACHED["nc"] = build_kernel()
    nc = _CACHED["nc"]
    in_maps = _prep_inputs(hidden_states, cos, sin, position_ids,
                           attention_mask, wq, wk, wv, wo)
    res = bass_utils.run_bass_kernel_spmd(
        nc, in_maps, core_ids=list(range(NCORES)), **run_kwargs)
    # core r's out rows: stq=0 -> seq[(r//4)*512 + (r%4)*128 : +128],
    #                    stq=1 -> seq[(2+r//4)*512 + (r%4)*128 : +128];
    # device layout is [stq, quad, 128, 1024]
    full = np.empty((S, HID), dtype=np.float32)
    for r in range(NCORES):
        o = np.asarray(res.results[r]["out"], dtype=np.float32)
        o = o.reshape(2, 4, 128, 1024).transpose(0, 2, 1, 3).reshape(
            2, 128, HID)
        b1 = (r // 4) * 512 + (r % 4) * 128
        b2 = (2 + r // 4) * 512 + (r % 4) * 128
        full[b1:b1 + 128] = o[0]
        full[b2:b2 + 128] = o[1]
    out = full.reshape(1, S, HID)
    if run_kwargs:
        _CACHED["last_result"] = res
    return out
